# revision 8
# baseline (speedup 1.0000x reference)
"""Mixture-of-Softmaxes kernel for 8 Trainium2 NeuronCores.

Strategy: tensor-parallel over the vocab dimension (V=100000 -> 12500/core).
Each core computes all B rows for its vocab shard: per-head logits via bf16
matmuls, exp via ScalarE (with free row-sum accumulation), a tiny [128,4]
AllReduce of the per-head softmax denominators across cores, then the
pi-weighted mixture on VectorE. Output is gathered on the host by
concatenating the vocab shards.

Host-side prep: inputs are transposed (contraction dim -> SBUF partitions)
and cast to bf16 before DMA, so the kernel needs no on-chip transposes.
"""

import numpy as np
import ml_dtypes

import concourse.bass as bass
import concourse.mybir as mybir
import concourse.tile as tile
from concourse import bacc
from concourse.bass_utils import run_bass_kernel_spmd
from concourse.bass_interp import get_hw_module

B, H, D, V = 1024, 4, 256, 100000
N_CORES = 8
V_S = V // N_CORES          # 12500 vocab entries per core
KT = D // 128               # 2 contraction k-tiles
BBLK = 128                  # b rows per block (= SBUF partitions)
N_BBLK = B // BBLK          # 8 blocks
VCHUNK = 2048               # psum chunk (4 banks); 12500 = 6*2048 + 212

F32 = mybir.dt.float32
BF16 = mybir.dt.bfloat16

_CHUNKS = []
_off = 0
while _off < V_S:
    _CHUNKS.append((_off, min(VCHUNK, V_S - _off)))
    _off += VCHUNK

_RUN_KWARGS = {}  # test harness may set trace/tmpdir here
_CACHE = {}


def _build():
    nc = bacc.Bacc("TRN2", target_bir_lowering=False, debug=False,
                   num_devices=N_CORES)
    xT = nc.dram_tensor("xT", [D, B], BF16, kind="ExternalInput").ap()
    pmT = nc.dram_tensor("pmT", [D, H * D], BF16, kind="ExternalInput").ap()
    mmT = nc.dram_tensor("mmT", [D, H], BF16, kind="ExternalInput").ap()
    embT = nc.dram_tensor("embT", [D, V_S], BF16, kind="ExternalInput").ap()
    out = nc.dram_tensor("out", [B, V_S], BF16, kind="ExternalOutput").ap()

    with tile.TileContext(nc) as tc:
        _body(tc, xT, pmT, mmT, embT, out)
        tc._pool_ctx.close()

    nc.compile()
    nc.m = get_hw_module(nc.m)
    return nc


def _body(tc, xT, pmT, mmT, embT, out):
    nc = tc.nc
    Exp = mybir.ActivationFunctionType.Exp
    Tanh = mybir.ActivationFunctionType.Tanh
    mul = mybir.AluOpType.mult
    add = mybir.AluOpType.add

    import contextlib
    ctx = contextlib.ExitStack()
    tc._pool_ctx = ctx
    singles = ctx.enter_context(tc.tile_pool(name="singles", bufs=1))
    work = ctx.enter_context(tc.tile_pool(name="work", bufs=2))
    outp = ctx.enter_context(tc.tile_pool(name="outp", bufs=1))
    psum = ctx.enter_context(tc.tile_pool(name="psum", bufs=2, space="PSUM"))
    dram = ctx.enter_context(tc.tile_pool(name="dram", bufs=2, space="DRAM"))

    # ---- resident SBUF inputs ----
    sb_xT = []
    sb_pmT = []
    sb_mmT = []
    sb_embT = []
    for k in range(KT):
        t = singles.tile([128, B], BF16, tag=f"xT{k}", name=f"xT{k}")
        nc.sync.dma_start(out=t, in_=xT[k * 128:(k + 1) * 128, :])
        sb_xT.append(t)
        t = singles.tile([128, H * D], BF16, tag=f"pmT{k}", name=f"pmT{k}")
        nc.sync.dma_start(out=t, in_=pmT[k * 128:(k + 1) * 128, :])
        sb_pmT.append(t)
        t = singles.tile([128, H], BF16, tag=f"mmT{k}", name=f"mmT{k}")
        nc.sync.dma_start(out=t, in_=mmT[k * 128:(k + 1) * 128, :])
        sb_mmT.append(t)
        t = singles.tile([128, V_S], BF16, tag=f"embT{k}", name=f"embT{k}")
        nc.sync.dma_start(out=t, in_=embT[k * 128:(k + 1) * 128, :])
        sb_embT.append(t)

    # ---- projT[h][kd] = tanh(proj_mat_h @ x.T) as [128 d, B b] bf16 ----
    sb_projT = [[singles.tile([128, B], BF16, tag=f"projT{h}_{kd}", name=f"projT{h}_{kd}")
                 for kd in range(KT)] for h in range(H)]
    for h in range(H):
        for kd in range(KT):
            for bs in range(B // 512):
                ps = psum.tile([128, VCHUNK], F32, tag="ps", name="ps")
                for kc in range(KT):
                    nc.tensor.matmul(
                        ps[:, :512],
                        sb_pmT[kc][:, h * D + kd * 128: h * D + (kd + 1) * 128],
                        sb_xT[kc][:, bs * 512:(bs + 1) * 512],
                        start=(kc == 0), stop=(kc == KT - 1),
                    )
                nc.scalar.activation(
                    out=sb_projT[h][kd][:, bs * 512:(bs + 1) * 512],
                    in_=ps[:, :512], func=Tanh,
                )

    # ---- pi[b, h] = softmax_h(x @ mix_mat.T) per b-block ----
    sb_pi = []
    for i in range(N_BBLK):
        ps = psum.tile([128, VCHUNK], F32, tag="ps", name="ps")
        for kc in range(KT):
            nc.tensor.matmul(
                ps[:, :H],
                sb_xT[kc][:, i * 128:(i + 1) * 128],
                sb_mmT[kc],
                start=(kc == 0), stop=(kc == KT - 1),
            )
        m = work.tile([128, 1], F32, tag="pim", name="pim")
        nc.vector.tensor_reduce(out=m, in_=ps[:, :H],
                                axis=mybir.AxisListType.X,
                                op=mybir.AluOpType.max)
        negm = work.tile([128, 1], F32, tag="pinegm", name="pinegm")
        nc.vector.tensor_scalar_mul(negm, m, -1.0)
        e = work.tile([128, H], F32, tag="pie", name="pie")
        nc.scalar.activation(out=e, in_=ps[:, :H], func=Exp, bias=negm)
        s = work.tile([128, 1], F32, tag="pis", name="pis")
        nc.vector.tensor_reduce(out=s, in_=e, axis=mybir.AxisListType.X,
                                op=add)
        rs = work.tile([128, 1], F32, tag="pirs", name="pirs")
        nc.vector.reciprocal(rs, s)
        pi = singles.tile([128, H], F32, tag=f"pi{i}", name=f"pi{i}")
        nc.vector.tensor_scalar_mul(pi, e, rs)
        sb_pi.append(pi)

    # ---- main loop over b-blocks ----
    # Per (block, head): matmul logits in psum chunks -> exp into resident
    # sb_e[h] (bf16) with row-sum side-accumulation -> per-head [128,1]
    # AllReduce issued as soon as the head finishes -> whole-tile DVE
    # mixture pass h chained into acc. The per-head split keeps the
    # collective latency off the Tensor/Scalar critical path: pass h's
    # collective completes while heads h+1.. are still exp-ing.
    sb_e = [singles.tile([128, V_S], BF16, tag=f"e{h}", name=f"e{h}") for h in range(H)]
    for i in range(N_BBLK):
        acc = outp.tile([128, V_S], BF16, tag="acc", name="acc")
        for h in range(H):
            sparts = work.tile([128, 8], F32, tag=f"sp{h}", name=f"sp{h}")
            for ci, (c0, cw) in enumerate(_CHUNKS):
                ps = psum.tile([128, VCHUNK], F32, tag="ps", name="ps")
                for kc in range(KT):
                    for ns in range((cw + 511) // 512):
                        n0 = ns * 512
                        nw = min(512, cw - n0)
                        nc.tensor.matmul(
                            ps[:, n0:n0 + nw],
                            sb_projT[h][kc][:, i * 128:(i + 1) * 128],
                            sb_embT[kc][:, c0 + n0:c0 + n0 + nw],
                            start=(kc == 0), stop=(kc == KT - 1),
                        )
                nc.scalar.activation(
                    out=sb_e[h][:, c0:c0 + cw], in_=ps[:, :cw], func=Exp,
                    accum_out=sparts[:, ci:ci + 1],
                )

            # head-h denominator -> AllReduce across vocab shards
            s_loc = work.tile([128, 1], F32, tag=f"sloc{h}", name=f"sloc{h}")
            nc.vector.tensor_reduce(
                out=s_loc, in_=sparts[:, :len(_CHUNKS)],
                axis=mybir.AxisListType.X, op=add)
            cc_in = dram.tile([128, 1], F32, tag=f"ccin{h}", name=f"ccin{h}")
            cc_out = dram.tile([128, 1], F32, tag=f"ccout{h}", name=f"ccout{h}")
            nc.gpsimd.dma_start(out=cc_in[:], in_=s_loc)
            nc.gpsimd.collective_compute(
                "AllReduce", add,
                replica_groups=[list(range(N_CORES))],
                ins=[cc_in.opt()], outs=[cc_out.opt()],
            )
            s_glob = work.tile([128, 1], F32, tag=f"sglob{h}", name=f"sglob{h}")
            nc.sync.dma_start(out=s_glob, in_=cc_out[:])
            rS = work.tile([128, 1], F32, tag=f"rS{h}", name=f"rS{h}")
            nc.vector.reciprocal(rS, s_glob)
            w = work.tile([128, 1], F32, tag=f"w{h}", name=f"w{h}")
            nc.vector.tensor_mul(w, sb_pi[i][:, h:h + 1], rS)

            # mixture pass h over the whole vocab shard (DVE, 2x bf16)
            if h == 0:
                nc.vector.tensor_scalar_mul(acc, sb_e[0], w)
            else:
                nc.vector.scalar_tensor_tensor(
                    out=acc, in0=sb_e[h], scalar=w, in1=acc,
                    op0=mul, op1=add)
        nc.sync.dma_start(out=out[i * 128:(i + 1) * 128, :], in_=acc)


def _get_nc():
    if "nc" not in _CACHE:
        _CACHE["nc"] = _build()
    return _CACHE["nc"]


def kernel(x, proj_mat, mix_mat, emb):
    nc = _get_nc()
    bf = ml_dtypes.bfloat16
    xT = np.ascontiguousarray(x.astype(bf).T)
    pmT = np.ascontiguousarray(proj_mat.astype(bf).T)
    mmT = np.ascontiguousarray(mix_mat.astype(bf).T)
    emb_bf = emb.astype(bf)
    in_maps = []
    for c in range(N_CORES):
        embT = np.ascontiguousarray(emb_bf[c * V_S:(c + 1) * V_S].T)
        in_maps.append({"xT": xT, "pmT": pmT, "mmT": mmT, "embT": embT})
    res = run_bass_kernel_spmd(nc, in_maps, list(range(N_CORES)),
                               **_RUN_KWARGS)
    _CACHE["last_result"] = res
    return np.concatenate(
        [res.results[c]["out"].astype(np.float32) for c in range(N_CORES)],
        axis=1)


# revision 10
# speedup vs baseline: 1.0355x; 1.0355x over previous
"""Mixture-of-Softmaxes kernel for 8 Trainium2 NeuronCores.

Strategy: tensor-parallel over the vocab dimension (V=100000 -> 12500/core).
Each core computes all B rows for its vocab shard: per-head logits via bf16
matmuls, exp via ScalarE (with free row-sum accumulation), a tiny [128,4]
AllReduce of the per-head softmax denominators across cores, then the
pi-weighted mixture on VectorE. Output is gathered on the host by
concatenating the vocab shards.

Host-side prep: inputs are transposed (contraction dim -> SBUF partitions)
and cast to bf16 before DMA, so the kernel needs no on-chip transposes.
"""

import numpy as np
import ml_dtypes

import concourse.bass as bass
import concourse.mybir as mybir
import concourse.tile as tile
from concourse import bacc
from concourse.bass_utils import run_bass_kernel_spmd
from concourse.bass_interp import get_hw_module

B, H, D, V = 1024, 4, 256, 100000
N_CORES = 8
V_S = V // N_CORES          # 12500 vocab entries per core
KT = D // 128               # 2 contraction k-tiles
BBLK = 128                  # b rows per block (= SBUF partitions)
N_BBLK = B // BBLK          # 8 blocks
VCHUNK = 2048               # psum chunk (4 banks); 12500 = 6*2048 + 212

F32 = mybir.dt.float32
BF16 = mybir.dt.bfloat16

_CHUNKS = []
_off = 0
while _off < V_S:
    _CHUNKS.append((_off, min(VCHUNK, V_S - _off)))
    _off += VCHUNK

QCHUNK = 3125  # mixture sub-tile (V_S/4)
_QUARTERS = []
_off = 0
while _off < V_S:
    _QUARTERS.append((_off, min(QCHUNK, V_S - _off)))
    _off += QCHUNK

_RUN_KWARGS = {}  # test harness may set trace/tmpdir here
_CACHE = {}


def _build():
    nc = bacc.Bacc("TRN2", target_bir_lowering=False, debug=False,
                   num_devices=N_CORES)
    xT = nc.dram_tensor("xT", [D, B], BF16, kind="ExternalInput").ap()
    pmT = nc.dram_tensor("pmT", [D, H * D], BF16, kind="ExternalInput").ap()
    mmT = nc.dram_tensor("mmT", [D, H], BF16, kind="ExternalInput").ap()
    embT = nc.dram_tensor("embT", [D, V_S], BF16, kind="ExternalInput").ap()
    out = nc.dram_tensor("out", [B, V_S], BF16, kind="ExternalOutput").ap()

    with tile.TileContext(nc) as tc:
        _body(tc, xT, pmT, mmT, embT, out)
        tc._pool_ctx.close()

    nc.compile()
    nc.m = get_hw_module(nc.m)
    return nc


def _body(tc, xT, pmT, mmT, embT, out):
    nc = tc.nc
    Exp = mybir.ActivationFunctionType.Exp
    Tanh = mybir.ActivationFunctionType.Tanh
    mul = mybir.AluOpType.mult
    add = mybir.AluOpType.add

    import contextlib
    ctx = contextlib.ExitStack()
    tc._pool_ctx = ctx
    singles = ctx.enter_context(tc.tile_pool(name="singles", bufs=1))
    work = ctx.enter_context(tc.tile_pool(name="work", bufs=2))
    outp = ctx.enter_context(tc.tile_pool(name="outp", bufs=1))
    psum = ctx.enter_context(tc.tile_pool(name="psum", bufs=2, space="PSUM"))
    dram = ctx.enter_context(tc.tile_pool(name="dram", bufs=2, space="DRAM"))

    # ---- resident SBUF inputs ----
    sb_xT = []
    sb_pmT = []
    sb_mmT = []
    sb_embT = []
    for k in range(KT):
        t = singles.tile([128, B], BF16, tag=f"xT{k}", name=f"xT{k}")
        nc.sync.dma_start(out=t, in_=xT[k * 128:(k + 1) * 128, :])
        sb_xT.append(t)
        t = singles.tile([128, H * D], BF16, tag=f"pmT{k}", name=f"pmT{k}")
        nc.sync.dma_start(out=t, in_=pmT[k * 128:(k + 1) * 128, :])
        sb_pmT.append(t)
        t = singles.tile([128, H], BF16, tag=f"mmT{k}", name=f"mmT{k}")
        nc.sync.dma_start(out=t, in_=mmT[k * 128:(k + 1) * 128, :])
        sb_mmT.append(t)
        t = singles.tile([128, V_S], BF16, tag=f"embT{k}", name=f"embT{k}")
        nc.sync.dma_start(out=t, in_=embT[k * 128:(k + 1) * 128, :])
        sb_embT.append(t)

    # ---- projT[h][kd] = tanh(proj_mat_h @ x.T) as [128 d, B b] bf16 ----
    sb_projT = [[singles.tile([128, B], BF16, tag=f"projT{h}_{kd}", name=f"projT{h}_{kd}")
                 for kd in range(KT)] for h in range(H)]
    for h in range(H):
        for kd in range(KT):
            for bs in range(B // 512):
                ps = psum.tile([128, VCHUNK], F32, tag="ps", name="ps")
                for kc in range(KT):
                    nc.tensor.matmul(
                        ps[:, :512],
                        sb_pmT[kc][:, h * D + kd * 128: h * D + (kd + 1) * 128],
                        sb_xT[kc][:, bs * 512:(bs + 1) * 512],
                        start=(kc == 0), stop=(kc == KT - 1),
                    )
                nc.scalar.activation(
                    out=sb_projT[h][kd][:, bs * 512:(bs + 1) * 512],
                    in_=ps[:, :512], func=Tanh,
                )

    # ---- pi[b, h] = softmax_h(x @ mix_mat.T) per b-block ----
    sb_pi = []
    for i in range(N_BBLK):
        ps = psum.tile([128, VCHUNK], F32, tag="ps", name="ps")
        for kc in range(KT):
            nc.tensor.matmul(
                ps[:, :H],
                sb_xT[kc][:, i * 128:(i + 1) * 128],
                sb_mmT[kc],
                start=(kc == 0), stop=(kc == KT - 1),
            )
        m = work.tile([128, 1], F32, tag="pim", name="pim")
        nc.vector.tensor_reduce(out=m, in_=ps[:, :H],
                                axis=mybir.AxisListType.X,
                                op=mybir.AluOpType.max)
        negm = work.tile([128, 1], F32, tag="pinegm", name="pinegm")
        nc.vector.tensor_scalar_mul(negm, m, -1.0)
        e = work.tile([128, H], F32, tag="pie", name="pie")
        nc.scalar.activation(out=e, in_=ps[:, :H], func=Exp, bias=negm)
        s = work.tile([128, 1], F32, tag="pis", name="pis")
        nc.vector.tensor_reduce(out=s, in_=e, axis=mybir.AxisListType.X,
                                op=add)
        rs = work.tile([128, 1], F32, tag="pirs", name="pirs")
        nc.vector.reciprocal(rs, s)
        pi = singles.tile([128, H], F32, tag=f"pi{i}", name=f"pi{i}")
        nc.vector.tensor_scalar_mul(pi, e, rs)
        sb_pi.append(pi)

    # ---- main loop over b-blocks ----
    # Per (block, head): matmul logits in psum chunks -> exp into resident
    # sb_e[h] (bf16) with row-sum side-accumulation -> per-head [128,1]
    # AllReduce issued as soon as the head finishes -> whole-tile DVE
    # mixture pass h chained into acc. The per-head split keeps the
    # collective latency off the Tensor/Scalar critical path: pass h's
    # collective completes while heads h+1.. are still exp-ing.
    sb_e = [singles.tile([128, V_S], BF16, tag=f"e{h}", name=f"e{h}") for h in range(H)]
    for i in range(N_BBLK):
        acc = outp.tile([128, V_S], BF16, tag="acc", name="acc")
        for h in range(H):
            sparts = work.tile([128, 8], F32, tag=f"sp{h}", name=f"sp{h}")
            for ci, (c0, cw) in enumerate(_CHUNKS):
                ps = psum.tile([128, VCHUNK], F32, tag="ps", name="ps")
                for kc in range(KT):
                    for ns in range((cw + 511) // 512):
                        n0 = ns * 512
                        nw = min(512, cw - n0)
                        nc.tensor.matmul(
                            ps[:, n0:n0 + nw],
                            sb_projT[h][kc][:, i * 128:(i + 1) * 128],
                            sb_embT[kc][:, c0 + n0:c0 + n0 + nw],
                            start=(kc == 0), stop=(kc == KT - 1),
                        )
                nc.scalar.activation(
                    out=sb_e[h][:, c0:c0 + cw], in_=ps[:, :cw], func=Exp,
                    accum_out=sparts[:, ci:ci + 1],
                )

            # head-h denominator -> AllReduce across vocab shards
            s_loc = work.tile([128, 1], F32, tag=f"sloc{h}", name=f"sloc{h}")
            nc.vector.tensor_reduce(
                out=s_loc, in_=sparts[:, :len(_CHUNKS)],
                axis=mybir.AxisListType.X, op=add)
            cc_in = dram.tile([128, 1], F32, tag=f"ccin{h}", name=f"ccin{h}")
            cc_out = dram.tile([128, 1], F32, tag=f"ccout{h}", name=f"ccout{h}")
            nc.gpsimd.dma_start(out=cc_in[:], in_=s_loc)
            nc.gpsimd.collective_compute(
                "AllReduce", add,
                replica_groups=[list(range(N_CORES))],
                ins=[cc_in.opt()], outs=[cc_out.opt()],
            )
            s_glob = work.tile([128, 1], F32, tag=f"sglob{h}", name=f"sglob{h}")
            nc.sync.dma_start(out=s_glob, in_=cc_out[:])
            rS = work.tile([128, 1], F32, tag=f"rS{h}", name=f"rS{h}")
            nc.vector.reciprocal(rS, s_glob)
            w = work.tile([128, 1], F32, tag=f"w{h}", name=f"w{h}")
            nc.vector.tensor_mul(w, sb_pi[i][:, h:h + 1], rS)

            # mixture pass h (DVE): tensor_scalar runs at 4x for bf16,
            # tensor_tensor at 2x; scalar_tensor_tensor would be 1x.
            for q0, qw in _QUARTERS:
                if h == 0:
                    nc.vector.tensor_scalar_mul(
                        acc[:, q0:q0 + qw], sb_e[0][:, q0:q0 + qw], w)
                else:
                    t1 = outp.tile([128, QCHUNK], BF16, tag="t1", name="t1")
                    nc.vector.tensor_scalar_mul(
                        t1[:, :qw], sb_e[h][:, q0:q0 + qw], w)
                    nc.vector.tensor_tensor(
                        out=acc[:, q0:q0 + qw], in0=acc[:, q0:q0 + qw],
                        in1=t1[:, :qw], op=add)
        nc.sync.dma_start(out=out[i * 128:(i + 1) * 128, :], in_=acc)


def _get_nc():
    if "nc" not in _CACHE:
        _CACHE["nc"] = _build()
    return _CACHE["nc"]


def kernel(x, proj_mat, mix_mat, emb):
    nc = _get_nc()
    bf = ml_dtypes.bfloat16
    xT = np.ascontiguousarray(x.astype(bf).T)
    pmT = np.ascontiguousarray(proj_mat.astype(bf).T)
    mmT = np.ascontiguousarray(mix_mat.astype(bf).T)
    emb_bf = emb.astype(bf)
    in_maps = []
    for c in range(N_CORES):
        embT = np.ascontiguousarray(emb_bf[c * V_S:(c + 1) * V_S].T)
        in_maps.append({"xT": xT, "pmT": pmT, "mmT": mmT, "embT": embT})
    res = run_bass_kernel_spmd(nc, in_maps, list(range(N_CORES)),
                               **_RUN_KWARGS)
    _CACHE["last_result"] = res
    return np.concatenate(
        [res.results[c]["out"].astype(np.float32) for c in range(N_CORES)],
        axis=1)
